# revision 2
# baseline (speedup 1.0000x reference)
"""AtomAttentionEncoder — 8-core SPMD kernel for trn2 (axon-tunneled NeuronCores).

Strategy (per spec sharding_hint): sequence-parallel over the atom (query)
dimension. Each of the 8 cores owns 192 atoms and carries a 192-atom halo on
each side (576-atom local region, 18 query blocks of 32). The 32x128
block-local attention mask means block g only attends keys [32g-48, 32g+80),
so the pair tensor plm is only materialized on those windows
([18, 32, 128, 16] per core instead of [1536, 1536, 16] global — 12x fewer
pair FLOPs). The halo lets every layer run with ZERO inter-core collectives
(validity shrinks 2 blocks/side/layer: 6-block margin covers L=3 layers);
the only collective is one psum at the final atom->token aggregation.
All arithmetic runs on the NeuronCores inside one pmap'd SPMD program;
host work is layout only (sharding slices, clamped halo indices, constant
block-geometry masks).
"""

import numpy as np
import jax
import jax.numpy as jnp
from functools import partial

B, N_ATOM, N_TOK = 1, 1536, 384
C_ATOM, C_PAIR, C_TOK = 128, 16, 384
C_HID, H, L = 32, 4, 3
HID = 2 * C_ATOM
N_QUERY, N_KEY, INF = 32, 128, 1e9

NCORES = 8
OWN = N_ATOM // NCORES          # 192 atoms owned per core
MARGIN = 192                    # halo per side (6 blocks; >= 64*L)
LOC = OWN + 2 * MARGIN          # 576-atom local region
NBLK = LOC // N_QUERY           # 18 local query blocks
PAD = 48                        # window overhang each side
OWN_LO, OWN_HI = MARGIN, MARGIN + OWN

# window gather index into the 48-padded local axis: key k of block g sits at
# padded index 32g + k  (window = [32g-48, 32g+80) in local coords)
_WIN_IDX = (32 * np.arange(NBLK)[:, None] + np.arange(N_KEY)[None, :])  # [18,128]


def _ln(x, gamma=None, beta=None, eps=1e-5):
    mu = jnp.mean(x, axis=-1, keepdims=True)
    var = jnp.mean(jnp.square(x - mu), axis=-1, keepdims=True)
    y = (x - mu) * jax.lax.rsqrt(var + eps)
    if gamma is not None:
        y = y * gamma
    if beta is not None:
        y = y + beta
    return y


def _adaln(a, s, gamma_s, Wg, bg, Ws):
    an = _ln(a)
    sn = _ln(s, gamma_s)
    return jax.nn.sigmoid(sn @ Wg + bg) * an + sn @ Ws


def _pad48(x):
    """pad the leading (atom) axis by 48 on each side with zeros."""
    cfg = [(PAD, PAD)] + [(0, 0)] * (x.ndim - 1)
    return jnp.pad(x, cfg)


def _win(x):
    """[LOC(+pad), ...] -> [NBLK, 128, ...] block-local key windows."""
    return _pad48(x)[_WIN_IDX]


@partial(jax.pmap, axis_name="x",
         in_axes=(0,) * 8 + (None,) * 35)
def _fwd(pos, msk, elem, chg, chars, uid, a2t, win_ok, tok_mask,
         W_feats, W_ref_offset, W_inv_sq, W_valid, W_l, W_m,
         W_mlp1, W_mlp2, W_mlp3, W_out_tok,
         attn_ada_gamma_s, attn_ada_Wg, attn_ada_bg, attn_ada_Ws,
         Wq, bq, Wk, Wv, lnz_g, lnz_b, Wb, Wgate, Wo, Wsg, bsg,
         tr_ada_gamma_s, tr_ada_Wg, tr_ada_bg, tr_ada_Ws,
         tr_W1, tr_W2, tr_Wog, tr_bog, tr_Wout):
    # ---- RefAtomFeatureEmbedder on the 576-atom local region ----
    feats = jnp.concatenate(
        [pos, msk[:, None], elem, chg[:, None],
         chars.reshape(LOC, -1), uid[:, None]], axis=-1)
    cl = feats @ W_feats                                    # [LOC, 128]

    pos_w = _win(pos)                                       # [18,128,3]
    uid_w = _win(uid)                                       # [18,128]
    pos_q = pos.reshape(NBLK, N_QUERY, 3)
    uid_q = uid.reshape(NBLK, N_QUERY)
    d = pos_w[:, None, :, :] - pos_q[:, :, None, :]         # [18,32,128,3]
    v = (uid_w[:, None, :] == uid_q[:, :, None]).astype(jnp.float32)[..., None]
    plm = (d @ W_ref_offset) * v
    inv_sq = 1.0 / (1.0 + jnp.sum(d * d, axis=-1, keepdims=True))
    plm = plm + (inv_sq @ W_inv_sq) * v + (v @ W_valid) * v  # [18,32,128,16]

    crelu = jax.nn.relu(cl)
    cr_l = (crelu @ W_l).reshape(NBLK, N_QUERY, 1, C_PAIR)
    cr_m = _win(crelu @ W_m)[:, None, :, :]                  # [18,1,128,16]
    plm = plm + cr_l + cr_m
    h = jax.nn.relu(plm) @ W_mlp1
    h = jax.nn.relu(h) @ W_mlp2
    h = jax.nn.relu(h) @ W_mlp3
    plm = plm + h

    # ---- attention masks (additive, exact -INF semantics of reference) ----
    atom_mask = a2t @ tok_mask                               # [LOC]
    keymask = _win(atom_mask) * win_ok                       # [18,128] in {0,1}
    addmask = (keymask - 1.0) * INF                          # 0 or -INF
    inv_sqrt = 1.0 / np.sqrt(C_HID)

    a = s = cl
    for i in range(L):
        an = _adaln(a, s, attn_ada_gamma_s[i], attn_ada_Wg[i],
                    attn_ada_bg[i], attn_ada_Ws[i])
        q = (an @ Wq[i] + bq[i]).reshape(NBLK, N_QUERY, H, C_HID)
        k = _win((an @ Wk[i]).reshape(LOC, H, C_HID))        # [18,128,H,32]
        vv = _win((an @ Wv[i]).reshape(LOC, H, C_HID))
        zb = _ln(plm, lnz_g[i], lnz_b[i]) @ Wb[i]            # [18,32,128,H]
        logits = (jnp.einsum('gqhc,gkhc->ghqk', q, k) * inv_sqrt
                  + jnp.moveaxis(zb, -1, 1)
                  + addmask[:, None, None, :])
        A = jax.nn.softmax(logits, axis=-1)
        o = jnp.einsum('ghqk,gkhc->gqhc', A, vv).reshape(LOC, H * C_HID)
        g = jax.nn.sigmoid(an @ Wgate[i])
        o = (g * o) @ Wo[i]
        attn_out = jax.nn.sigmoid(s @ Wsg[i] + bsg[i]) * o
        tn = _adaln(a, s, tr_ada_gamma_s[i], tr_ada_Wg[i],
                    tr_ada_bg[i], tr_ada_Ws[i])
        hh = jax.nn.silu(tn @ tr_W1[i]) * (tn @ tr_W2[i])
        tr_out = jax.nn.sigmoid(s @ tr_Wog[i] + tr_bog[i]) * (hh @ tr_Wout[i])
        a = attn_out + tr_out

    # ---- atom -> token mean-aggregation (owned slice only; one AllReduce) ----
    al = jax.nn.relu(a[OWN_LO:OWN_HI] @ W_out_tok)           # [192, 384]
    a2t_own = a2t[OWN_LO:OWN_HI]                             # [192, 384]
    part = a2t_own.T @ al                                    # [384, 384]
    cnt = jnp.sum(a2t_own, axis=0)                           # [384]
    tot = jax.lax.psum(jnp.concatenate([part, cnt[None, :]], axis=0), "x")
    return tot[:N_TOK] / jnp.maximum(tot[N_TOK], 1.0)[:, None]


def kernel(**inputs):
    inp = {k: np.asarray(v) for k, v in inputs.items()}

    # per-core halo shards (layout only: clamped-index slicing)
    starts = np.arange(NCORES) * OWN - MARGIN
    idx = np.clip(starts[:, None] + np.arange(LOC)[None, :], 0, N_ATOM - 1)

    def shard(x):  # x: [1, N_ATOM, ...] -> [8, LOC, ...]
        return x[0][idx]

    # constant geometry mask: key global index in [0, N_ATOM)
    gk = (starts[:, None, None] + 32 * np.arange(NBLK)[None, :, None] - PAD
          + np.arange(N_KEY)[None, None, :])
    win_ok = ((gk >= 0) & (gk < N_ATOM)).astype(np.float32)   # [8,18,128]

    sharded = [shard(inp[n]) for n in
               ('ref_pos', 'ref_mask', 'ref_element', 'ref_charge',
                'ref_atom_name_chars', 'ref_space_uid', 'atom_to_token_index')]
    rep = [inp['token_mask'][0],
           inp['W_feats'], inp['W_ref_offset'], inp['W_inv_sq'],
           inp['W_valid'], inp['W_l'], inp['W_m'], inp['W_mlp1'],
           inp['W_mlp2'], inp['W_mlp3'], inp['W_out_tok'],
           inp['attn_ada_gamma_s'], inp['attn_ada_Wg'], inp['attn_ada_bg'],
           inp['attn_ada_Ws'], inp['Wq'], inp['bq'], inp['Wk'], inp['Wv'],
           inp['lnz_g'], inp['lnz_b'], inp['Wb'], inp['Wgate'], inp['Wo'],
           inp['Wsg'], inp['bsg'], inp['tr_ada_gamma_s'], inp['tr_ada_Wg'],
           inp['tr_ada_bg'], inp['tr_ada_Ws'], inp['tr_W1'], inp['tr_W2'],
           inp['tr_Wog'], inp['tr_bog'], inp['tr_Wout']]

    out = _fwd(*sharded, win_ok, rep[0], *rep[1:])
    return np.asarray(out[0])[None].astype(np.float32)       # [1, 384, 384]


# revision 7
# speedup vs baseline: 26.7179x; 26.7179x over previous
"""AtomAttentionEncoder — 8-core SPMD kernel for trn2 (axon-tunneled NeuronCores).

Strategy (per spec sharding_hint): sequence-parallel over the atom (query)
dimension. Each of the 8 cores owns 192 atoms and carries a 192-atom halo on
each side (576-atom local region, 18 query blocks of 32). The 32x128
block-local attention mask means block g only attends keys [32g-48, 32g+80),
so the pair tensor plm is only materialized on those windows
([18, 32, 128, 16] per core instead of [1536, 1536, 16] global — 12x fewer
pair FLOPs). The halo lets every layer run with ZERO inter-core collectives
(validity shrinks 2 blocks/side/layer: 6-block margin covers L=3 layers);
the only collective is one psum at the final atom->token aggregation.
All arithmetic runs on the NeuronCores inside one pmap'd SPMD program;
host work is layout only (sharding slices, clamped halo indices, constant
block-geometry masks).
"""

import numpy as np
import jax
import jax.numpy as jnp
from functools import partial

B, N_ATOM, N_TOK = 1, 1536, 384
C_ATOM, C_PAIR, C_TOK = 128, 16, 384
C_HID, H, L = 32, 4, 3
HID = 2 * C_ATOM
N_QUERY, N_KEY, INF = 32, 128, 1e9

NCORES = 8
OWN = N_ATOM // NCORES          # 192 atoms owned per core
MARGIN = 192                    # halo per side (6 blocks; >= 64*L)
LOC = OWN + 2 * MARGIN          # 576-atom local region
NBLK = LOC // N_QUERY           # 18 local query blocks
PAD = 48                        # window overhang each side
OWN_LO, OWN_HI = MARGIN, MARGIN + OWN

# window gather index into the 48-padded local axis: key k of block g sits at
# padded index 32g + k  (window = [32g-48, 32g+80) in local coords)
_WIN_IDX = (32 * np.arange(NBLK)[:, None] + np.arange(N_KEY)[None, :])  # [18,128]


def _ln(x, gamma=None, beta=None, eps=1e-5):
    mu = jnp.mean(x, axis=-1, keepdims=True)
    var = jnp.mean(jnp.square(x - mu), axis=-1, keepdims=True)
    y = (x - mu) * jax.lax.rsqrt(var + eps)
    if gamma is not None:
        y = y * gamma
    if beta is not None:
        y = y + beta
    return y


def _adaln(a, s, gamma_s, Wg, bg, Ws):
    an = _ln(a)
    sn = _ln(s, gamma_s)
    return jax.nn.sigmoid(sn @ Wg + bg) * an + sn @ Ws


def _pad48(x):
    """pad the leading (atom) axis by 48 on each side with zeros."""
    cfg = [(PAD, PAD)] + [(0, 0)] * (x.ndim - 1)
    return jnp.pad(x, cfg)


def _win(x):
    """[LOC(+pad), ...] -> [NBLK, 128, ...] block-local key windows."""
    return _pad48(x)[_WIN_IDX]


def _fwd_body(pos, msk, elem, chg, chars, uid, a2t, win_ok, tok_mask,
         W_feats, W_ref_offset, W_inv_sq, W_valid, W_l, W_m,
         W_mlp1, W_mlp2, W_mlp3, W_out_tok,
         attn_ada_gamma_s, attn_ada_Wg, attn_ada_bg, attn_ada_Ws,
         Wq, bq, Wk, Wv, lnz_g, lnz_b, Wb, Wgate, Wo, Wsg, bsg,
         tr_ada_gamma_s, tr_ada_Wg, tr_ada_bg, tr_ada_Ws,
         tr_W1, tr_W2, tr_Wog, tr_bog, tr_Wout):
    # ---- RefAtomFeatureEmbedder on the 576-atom local region ----
    feats = jnp.concatenate(
        [pos, msk[:, None], elem, chg[:, None],
         chars.reshape(LOC, -1), uid[:, None]], axis=-1)
    cl = feats @ W_feats                                    # [LOC, 128]

    pos_w = _win(pos)                                       # [18,128,3]
    uid_w = _win(uid)                                       # [18,128]
    pos_q = pos.reshape(NBLK, N_QUERY, 3)
    uid_q = uid.reshape(NBLK, N_QUERY)
    d = pos_w[:, None, :, :] - pos_q[:, :, None, :]         # [18,32,128,3]
    v = (uid_w[:, None, :] == uid_q[:, :, None]).astype(jnp.float32)[..., None]
    plm = (d @ W_ref_offset) * v
    inv_sq = 1.0 / (1.0 + jnp.sum(d * d, axis=-1, keepdims=True))
    plm = plm + (inv_sq @ W_inv_sq) * v + (v @ W_valid) * v  # [18,32,128,16]

    crelu = jax.nn.relu(cl)
    cr_l = (crelu @ W_l).reshape(NBLK, N_QUERY, 1, C_PAIR)
    cr_m = _win(crelu @ W_m)[:, None, :, :]                  # [18,1,128,16]
    plm = plm + cr_l + cr_m
    h = jax.nn.relu(plm) @ W_mlp1
    h = jax.nn.relu(h) @ W_mlp2
    h = jax.nn.relu(h) @ W_mlp3
    plm = plm + h

    # ---- attention masks (additive, exact -INF semantics of reference) ----
    atom_mask = a2t @ tok_mask                               # [LOC]
    keymask = _win(atom_mask) * win_ok                       # [18,128] in {0,1}
    addmask = (keymask - 1.0) * INF                          # 0 or -INF
    inv_sqrt = 1.0 / np.sqrt(C_HID)

    a = s = cl
    for i in range(L):
        an = _adaln(a, s, attn_ada_gamma_s[i], attn_ada_Wg[i],
                    attn_ada_bg[i], attn_ada_Ws[i])
        q = (an @ Wq[i] + bq[i]).reshape(NBLK, N_QUERY, H, C_HID)
        k = _win((an @ Wk[i]).reshape(LOC, H, C_HID))        # [18,128,H,32]
        vv = _win((an @ Wv[i]).reshape(LOC, H, C_HID))
        zb = _ln(plm, lnz_g[i], lnz_b[i]) @ Wb[i]            # [18,32,128,H]
        logits = (jnp.einsum('gqhc,gkhc->ghqk', q, k) * inv_sqrt
                  + jnp.moveaxis(zb, -1, 1)
                  + addmask[:, None, None, :])
        A = jax.nn.softmax(logits, axis=-1)
        o = jnp.einsum('ghqk,gkhc->gqhc', A, vv).reshape(LOC, H * C_HID)
        g = jax.nn.sigmoid(an @ Wgate[i])
        o = (g * o) @ Wo[i]
        attn_out = jax.nn.sigmoid(s @ Wsg[i] + bsg[i]) * o
        tn = _adaln(a, s, tr_ada_gamma_s[i], tr_ada_Wg[i],
                    tr_ada_bg[i], tr_ada_Ws[i])
        hh = jax.nn.silu(tn @ tr_W1[i]) * (tn @ tr_W2[i])
        tr_out = jax.nn.sigmoid(s @ tr_Wog[i] + tr_bog[i]) * (hh @ tr_Wout[i])
        a = attn_out + tr_out

    # ---- atom -> token mean-aggregation (owned slice only; one AllReduce) ----
    al = jax.nn.relu(a[OWN_LO:OWN_HI] @ W_out_tok)           # [192, 384]
    a2t_own = a2t[OWN_LO:OWN_HI]                             # [192, 384]
    part = a2t_own.T @ al                                    # [384, 384]
    cnt = jnp.sum(a2t_own, axis=0)                           # [384]
    tot = jax.lax.psum(jnp.concatenate([part, cnt[None, :]], axis=0), "x")
    return tot[:N_TOK] / jnp.maximum(tot[N_TOK], 1.0)[:, None]


_fwd = jax.pmap(_fwd_body, axis_name="x", in_axes=(0,) * 8 + (None,) * 35)


def _prep(inputs):
    """Host-side layout: halo shards + constant geometry masks."""
    inp = {k: np.asarray(v) for k, v in inputs.items()}

    # per-core halo shards (layout only: clamped-index slicing)
    starts = np.arange(NCORES) * OWN - MARGIN
    idx = np.clip(starts[:, None] + np.arange(LOC)[None, :], 0, N_ATOM - 1)

    def shard(x):  # x: [1, N_ATOM, ...] -> [8, LOC, ...]
        return x[0][idx]

    # constant geometry mask: key global index in [0, N_ATOM)
    gk = (starts[:, None, None] + 32 * np.arange(NBLK)[None, :, None] - PAD
          + np.arange(N_KEY)[None, None, :])
    win_ok = ((gk >= 0) & (gk < N_ATOM)).astype(np.float32)   # [8,18,128]

    sharded = [shard(inp[n]) for n in
               ('ref_pos', 'ref_mask', 'ref_element', 'ref_charge',
                'ref_atom_name_chars', 'ref_space_uid', 'atom_to_token_index')]
    rep = [inp['token_mask'][0],
           inp['W_feats'], inp['W_ref_offset'], inp['W_inv_sq'],
           inp['W_valid'], inp['W_l'], inp['W_m'], inp['W_mlp1'],
           inp['W_mlp2'], inp['W_mlp3'], inp['W_out_tok'],
           inp['attn_ada_gamma_s'], inp['attn_ada_Wg'], inp['attn_ada_bg'],
           inp['attn_ada_Ws'], inp['Wq'], inp['bq'], inp['Wk'], inp['Wv'],
           inp['lnz_g'], inp['lnz_b'], inp['Wb'], inp['Wgate'], inp['Wo'],
           inp['Wsg'], inp['bsg'], inp['tr_ada_gamma_s'], inp['tr_ada_Wg'],
           inp['tr_ada_bg'], inp['tr_ada_Ws'], inp['tr_W1'], inp['tr_W2'],
           inp['tr_Wog'], inp['tr_bog'], inp['tr_Wout']]

    return sharded + [win_ok] + rep


def kernel(**inputs):
    args = _prep(inputs)
    out = _fwd(*args)
    return np.asarray(out[0])[None].astype(np.float32)       # [1, 384, 384]


def stage(**inputs):
    """Pre-stage shards/weights on the 8 devices (for device-time benchmarks)."""
    args = _prep(inputs)
    devs = jax.devices()[:NCORES]
    staged = []
    for i, a in enumerate(args):
        if i < 8:  # sharded leading-8 args
            staged.append(jax.device_put_sharded(list(a), devs))
        else:
            staged.append(jax.device_put_replicated(a, devs))
    return staged


def run_staged(staged):
    return _fwd_staged(*staged)


@partial(jax.pmap, axis_name="x")
def _fwd_staged(*args):
    return _fwd_body(*args)


# revision 13
# speedup vs baseline: 310.2423x; 11.6118x over previous
"""AtomAttentionEncoder — 8-core SPMD kernel for trn2 (axon-tunneled NeuronCores).

Strategy (per spec sharding_hint): sequence-parallel over the atom (query)
dimension. Each of the 8 cores owns 192 atoms and carries a 192-atom halo on
each side (576-atom local region, 18 query blocks of 32). The 32x128
block-local attention mask means block g only attends keys [32g-48, 32g+80),
so the pair tensor plm is only materialized on those windows
([18, 32, 128, 16] per core instead of [1536, 1536, 16] global — 12x fewer
pair FLOPs). The halo lets every layer run with ZERO inter-core collectives
(validity shrinks 2 blocks/side/layer: 6-block margin covers L=3 layers);
the only collective is one psum at the final atom->token aggregation.
All arithmetic runs on the NeuronCores inside one pmap'd SPMD program;
host work is layout only (sharding slices, clamped halo indices, constant
block-geometry masks).
"""

import numpy as np
import jax
import jax.numpy as jnp
from functools import partial

B, N_ATOM, N_TOK = 1, 1536, 384
C_ATOM, C_PAIR, C_TOK = 128, 16, 384
C_HID, H, L = 32, 4, 3
HID = 2 * C_ATOM
N_QUERY, N_KEY, INF = 32, 128, 1e9

NCORES = 8
OWN = N_ATOM // NCORES          # 192 atoms owned per core
MARGIN = 192                    # halo per side (6 blocks; >= 64*L)
LOC = OWN + 2 * MARGIN          # 576-atom local region
NBLK = LOC // N_QUERY           # 18 local query blocks
PAD = 48                        # window overhang each side
OWN_LO, OWN_HI = MARGIN, MARGIN + OWN

# window gather index into the 48-padded local axis: key k of block g sits at
# padded index 32g + k  (window = [32g-48, 32g+80) in local coords)
_WIN_IDX = (32 * np.arange(NBLK)[:, None] + np.arange(N_KEY)[None, :])  # [18,128]


def _mm(x, w):
    """bf16 matmul with fp32 accumulation (PE runs bf16 at 4x fp32 rate)."""
    return jnp.matmul(x.astype(jnp.bfloat16), w.astype(jnp.bfloat16),
                      preferred_element_type=jnp.float32)


def _ein(eq, a, b):
    return jnp.einsum(eq, a.astype(jnp.bfloat16), b.astype(jnp.bfloat16),
                      preferred_element_type=jnp.float32)


def _ln(x, gamma=None, beta=None, eps=1e-5):
    mu = jnp.mean(x, axis=-1, keepdims=True)
    var = jnp.mean(jnp.square(x - mu), axis=-1, keepdims=True)
    y = (x - mu) * jax.lax.rsqrt(var + eps)
    if gamma is not None:
        y = y * gamma
    if beta is not None:
        y = y + beta
    return y


def _adaln(a, s, gamma_s, Wg, bg, Ws):
    an = _ln(a)
    sn = _ln(s, gamma_s)
    return jax.nn.sigmoid(_mm(sn, Wg) + bg) * an + _mm(sn, Ws)


def _pad48(x):
    """pad the leading (atom) axis by 48 on each side with zeros."""
    cfg = [(PAD, PAD)] + [(0, 0)] * (x.ndim - 1)
    return jnp.pad(x, cfg)


def _win(x):
    """[LOC(+pad), ...] -> [NBLK, 128, ...] block-local key windows."""
    return _pad48(x)[_WIN_IDX]


def _fwd_body(pos, msk, elem, chg, chars, uid, a2t, win_ok, tok_mask,
         W_feats, W_ref_offset, W_inv_sq, W_valid, W_l, W_m,
         W_mlp1, W_mlp2, W_mlp3, W_out_tok,
         attn_ada_gamma_s, attn_ada_Wg, attn_ada_bg, attn_ada_Ws,
         Wq, bq, Wk, Wv, lnz_g, lnz_b, Wb, Wgate, Wo, Wsg, bsg,
         tr_ada_gamma_s, tr_ada_Wg, tr_ada_bg, tr_ada_Ws,
         tr_W1, tr_W2, tr_Wog, tr_bog, tr_Wout):
    # ---- RefAtomFeatureEmbedder on the 576-atom local region ----
    feats = jnp.concatenate(
        [pos, msk[:, None], elem, chg[:, None],
         chars.reshape(LOC, -1), uid[:, None]], axis=-1)
    cl = _mm(feats, W_feats)                                # [LOC, 128]

    pos_w = _win(pos)                                       # [18,128,3]
    uid_w = _win(uid)                                       # [18,128]
    pos_q = pos.reshape(NBLK, N_QUERY, 3)
    uid_q = uid.reshape(NBLK, N_QUERY)
    d = pos_w[:, None, :, :] - pos_q[:, :, None, :]         # [18,32,128,3]
    v = (uid_w[:, None, :] == uid_q[:, :, None]).astype(jnp.float32)[..., None]
    plm = (d @ W_ref_offset) * v
    inv_sq = 1.0 / (1.0 + jnp.sum(d * d, axis=-1, keepdims=True))
    plm = plm + (inv_sq @ W_inv_sq) * v + (v @ W_valid) * v  # [18,32,128,16]

    crelu = jax.nn.relu(cl)
    cr_l = _mm(crelu, W_l).reshape(NBLK, N_QUERY, 1, C_PAIR)
    cr_m = _win(_mm(crelu, W_m))[:, None, :, :]              # [18,1,128,16]
    plm = plm + cr_l + cr_m
    h = _mm(jax.nn.relu(plm), W_mlp1)
    h = _mm(jax.nn.relu(h), W_mlp2)
    h = _mm(jax.nn.relu(h), W_mlp3)
    plm = plm + h

    # ---- attention masks (additive, exact -INF semantics of reference) ----
    atom_mask = a2t @ tok_mask                               # [LOC]
    keymask = _win(atom_mask) * win_ok                       # [18,128] in {0,1}
    addmask = (keymask - 1.0) * INF                          # 0 or -INF
    inv_sqrt = 1.0 / np.sqrt(C_HID)

    a = s = cl
    for i in range(L):
        an = _adaln(a, s, attn_ada_gamma_s[i], attn_ada_Wg[i],
                    attn_ada_bg[i], attn_ada_Ws[i])
        q = (_mm(an, Wq[i]) + bq[i]).reshape(NBLK, N_QUERY, H, C_HID)
        k = _win(_mm(an, Wk[i]).reshape(LOC, H, C_HID))      # [18,128,H,32]
        vv = _win(_mm(an, Wv[i]).reshape(LOC, H, C_HID))
        zb = _mm(_ln(plm, lnz_g[i], lnz_b[i]), Wb[i])        # [18,32,128,H]
        logits = (_ein('gqhc,gkhc->ghqk', q, k) * inv_sqrt
                  + jnp.moveaxis(zb, -1, 1)
                  + addmask[:, None, None, :])
        A = jax.nn.softmax(logits, axis=-1)
        o = _ein('ghqk,gkhc->gqhc', A, vv).reshape(LOC, H * C_HID)
        g = jax.nn.sigmoid(_mm(an, Wgate[i]))
        o = _mm(g * o, Wo[i])
        attn_out = jax.nn.sigmoid(_mm(s, Wsg[i]) + bsg[i]) * o
        tn = _adaln(a, s, tr_ada_gamma_s[i], tr_ada_Wg[i],
                    tr_ada_bg[i], tr_ada_Ws[i])
        hh = jax.nn.silu(_mm(tn, tr_W1[i])) * _mm(tn, tr_W2[i])
        tr_out = jax.nn.sigmoid(_mm(s, tr_Wog[i]) + tr_bog[i]) * _mm(hh, tr_Wout[i])
        a = attn_out + tr_out

    # ---- atom -> token mean-aggregation (owned slice only; one AllReduce) ----
    al = jax.nn.relu(_mm(a[OWN_LO:OWN_HI], W_out_tok))       # [192, 384]
    a2t_own = a2t[OWN_LO:OWN_HI]                             # [192, 384]
    part = a2t_own.T @ al                                    # [384, 384]
    cnt = jnp.sum(a2t_own, axis=0)                           # [384]
    tot = jax.lax.psum(jnp.concatenate([part, cnt[None, :]], axis=0), "x")
    return tot[:N_TOK] / jnp.maximum(tot[N_TOK], 1.0)[:, None]


_fwd = jax.pmap(_fwd_body, axis_name="x", in_axes=(0,) * 8 + (None,) * 35)


def _prep(inputs):
    """Host-side layout: halo shards + constant geometry masks."""
    inp = {k: np.asarray(v) for k, v in inputs.items()}

    # per-core halo shards (layout only: clamped-index slicing)
    starts = np.arange(NCORES) * OWN - MARGIN
    idx = np.clip(starts[:, None] + np.arange(LOC)[None, :], 0, N_ATOM - 1)

    def shard(x):  # x: [1, N_ATOM, ...] -> [8, LOC, ...]
        return x[0][idx]

    # constant geometry mask: key global index in [0, N_ATOM)
    gk = (starts[:, None, None] + 32 * np.arange(NBLK)[None, :, None] - PAD
          + np.arange(N_KEY)[None, None, :])
    win_ok = ((gk >= 0) & (gk < N_ATOM)).astype(np.float32)   # [8,18,128]

    sharded = [shard(inp[n]) for n in
               ('ref_pos', 'ref_mask', 'ref_element', 'ref_charge',
                'ref_atom_name_chars', 'ref_space_uid', 'atom_to_token_index')]
    rep = [inp['token_mask'][0],
           inp['W_feats'], inp['W_ref_offset'], inp['W_inv_sq'],
           inp['W_valid'], inp['W_l'], inp['W_m'], inp['W_mlp1'],
           inp['W_mlp2'], inp['W_mlp3'], inp['W_out_tok'],
           inp['attn_ada_gamma_s'], inp['attn_ada_Wg'], inp['attn_ada_bg'],
           inp['attn_ada_Ws'], inp['Wq'], inp['bq'], inp['Wk'], inp['Wv'],
           inp['lnz_g'], inp['lnz_b'], inp['Wb'], inp['Wgate'], inp['Wo'],
           inp['Wsg'], inp['bsg'], inp['tr_ada_gamma_s'], inp['tr_ada_Wg'],
           inp['tr_ada_bg'], inp['tr_ada_Ws'], inp['tr_W1'], inp['tr_W2'],
           inp['tr_Wog'], inp['tr_bog'], inp['tr_Wout']]

    return sharded + [win_ok] + rep


def kernel(**inputs):
    args = _prep(inputs)
    out = _fwd(*args)
    return np.asarray(out[0])[None].astype(np.float32)       # [1, 384, 384]


def stage(**inputs):
    """Pre-stage shards/weights on the 8 devices (for device-time benchmarks)."""
    args = _prep(inputs)
    devs = jax.devices()[:NCORES]
    staged = []
    for i, a in enumerate(args):
        if i < 8:  # sharded leading-8 args
            staged.append(jax.device_put_sharded(list(a), devs))
        else:
            staged.append(jax.device_put_replicated(a, devs))
    return staged


def run_staged(staged):
    return _fwd_staged(*staged)


@partial(jax.pmap, axis_name="x")
def _fwd_staged(*args):
    return _fwd_body(*args)
